# revision 61
# baseline (speedup 1.0000x reference)
"""DeepfakeGNN (2x GCNConv + mean-pool + fc) on 8 Trainium2 NeuronCores.

Scatter/ReduceScatter dataflow: graphs are split 16-per-core (batch is sorted,
so each core owns a contiguous node range). Edges are assigned to the core
owning their SRC node. Per layer: each core computes z = H @ W for its own
nodes (bf16), scatters edge messages into full-size partial accumulators via
dma_gather (rows by local src index) + one-hot segment matmuls on the tensor
engine, then ReduceScatter(add) returns each core the summed aggregation for
its own rows. The scatter/RS is split into two phases (A = first TA local
tiles of every core, B = the rest) so RS-A overlaps the B scatter and the
post-RS work of A overlaps RS-B. Bias is accumulated in-scatter as a closing
ones x (bias/8) matmul; self-loops are applied post-RS as a per-row scaled
copy of local z. Pool + fc are local; the host concatenates the per-core [16]
outputs.

Self-contained: only numpy + ml_dtypes + concourse (preinstalled).
"""
import numpy as np
import ml_dtypes

import concourse.mybir as mybir
from concourse import bacc
from concourse.bass_utils import run_bass_kernel_spmd
from concourse.masks import make_identity
from concourse.tile import TileContext

NC = 8          # cores
N = 20000       # nodes
D_IN = 512
DH = 256
G = 128         # graphs
GP = G // NC    # graphs per core

FP32 = mybir.dt.float32
BF16 = mybir.dt.bfloat16
I16 = mybir.dt.int16
I32 = mybir.dt.int32

BF = ml_dtypes.bfloat16


def _ta(T):
    """A-phase size: RS-A must hide under the B scatter."""
    return max(1, min(T - 1, (T + 1) // 2 + 1))


def _assign_graphs(gcnt):
    """Assign 16 graphs per core, balancing total node count (greedy LPT).
    Returns [NC][GP] graph ids."""
    order = np.argsort(-gcnt, kind="stable")
    totals = np.zeros(NC, dtype=np.int64)
    counts = np.zeros(NC, dtype=np.int64)
    bins = [[] for _ in range(NC)]
    for g in order:
        c = min((c for c in range(NC) if counts[c] < GP),
                key=lambda c: totals[c])
        bins[c].append(int(g))
        totals[c] += gcnt[g]
        counts[c] += 1
    return bins


def _pack_tiles(vecs, T):
    """Greedy-pack nodes (rows of vecs [n, NC] = per-src-core in-edge counts)
    into T tiles of <=128 nodes, minimizing the max per-core count per tile
    (target <= 256 so every tile needs only 2 chunks). Returns pos[n]."""
    n = vecs.shape[0]
    order = np.argsort(-vecs.sum(axis=1), kind="stable")
    sums = np.zeros((T, NC), dtype=np.int64)
    cnts = np.zeros(T, dtype=np.int64)
    pos = np.zeros(n, dtype=np.int64)
    for i in order:
        v = vecs[i]
        score = np.max(sums + v[None, :], axis=1).astype(np.float64)
        score[cnts >= 128] = np.inf
        t = int(np.argmin(score))
        pos[i] = t * 128 + cnts[t]
        sums[t] += v
        cnts[t] += 1
    return pos


# ---------------------------------------------------------------- host prep

def _wrap16(arr, cols):
    """Lay out a flat int array [cols*16] -> [128, cols] in dma_gather idx
    order (idx j at [j%16, j//16], replicated across the 8 q7 cores)."""
    a = arr.reshape(cols, 16).T  # [16, cols]
    return np.ascontiguousarray(np.tile(a, (8, 1)))


def prep(x, edge_index, batch, W1, b1, W2, b2, w_fc, b_fc):
    x = np.asarray(x, dtype=np.float32)
    ei = np.asarray(edge_index).astype(np.int64)
    batch = np.asarray(batch).astype(np.int64)
    W1 = np.asarray(W1, dtype=np.float32)
    b1 = np.asarray(b1, dtype=np.float32)
    W2 = np.asarray(W2, dtype=np.float32)
    b2 = np.asarray(b2, dtype=np.float32)
    w_fc = np.asarray(w_fc, dtype=np.float32)
    b_fc = np.asarray(b_fc, dtype=np.float32)

    n = x.shape[0]
    src, dst = ei[0], ei[1]

    # degree includes self-loop (reference concatenates loops)
    deg = (np.bincount(dst, minlength=n) + 1.0).astype(np.float32)
    dinv = (1.0 / np.sqrt(deg, dtype=np.float32)).astype(np.float32)
    coef = (dinv[src] * dinv[dst]).astype(np.float32)
    selfc = (dinv * dinv).astype(np.float32)

    # graphs -> cores balanced by node count; nodes follow their graph
    gcnt = np.bincount(batch, minlength=G).astype(np.int64)
    bins = _assign_graphs(gcnt)
    core_of_graph = np.zeros(G, dtype=np.int64)
    for c in range(NC):
        core_of_graph[bins[c]] = c
    owner = core_of_graph[batch]                  # owning core per node
    nodes_of = [np.where(owner == c)[0] for c in range(NC)]
    n_c = np.array([len(v) for v in nodes_of])
    # one extra tile of headroom so the packer can keep every tile at 2 chunks
    n_pad = int((int(np.ceil(n_c.max() / 128.0)) + 1) * 128)
    T = n_pad // 128
    TA = _ta(T)

    # Permute nodes within each core's block so that each 128-row dst tile
    # draws <=256 in-edges from every src core (2 chunks per tile).
    vec = np.zeros((n, NC), dtype=np.int64)
    np.add.at(vec, (dst, owner[src]), 1)
    newpos = np.zeros(n, dtype=np.int64)
    for c in range(NC):
        newpos[nodes_of[c]] = _pack_tiles(vec[nodes_of[c]], T)

    gd = owner * n_pad + newpos                   # global padded slot

    # processing order: phase A tiles (t < TA, c-major), then phase B
    order = [(c, t) for rng in (range(TA), range(TA, T))
             for c in range(NC) for t in rng]

    # per-core edge streams (src owned by core), grouped by global dst tile
    per_core = []
    for c in range(NC):
        m = owner[src] == c
        es = newpos[src[m]]                          # local z row (permuted)
        gde = gd[dst[m]]
        ec = coef[m]
        o = np.argsort(gde, kind="stable")
        es, ec, gde = es[o], ec[o], gde[o]
        tb = np.searchsorted(gde, np.arange(0, NC * n_pad + 1, 128))
        per_core.append((es, ec, gde, tb))

    # shared chunk schedule in processing order
    CH = []
    for (c_t) in order:
        cc, tt = c_t
        g = cc * T + tt
        mx = 1
        for c in range(NC):
            tb = per_core[c][3]
            cnt = int(tb[g + 1] - tb[g])
            mx = max(mx, (cnt + 127) // 128)
        CH.append(mx)
    TOT = sum(CH)

    # per-graph 1/count for mean pooling
    gcnt = np.bincount(batch, minlength=G).astype(np.float32)
    ginv = 1.0 / np.maximum(gcnt, 1.0)

    in_maps = []
    for c in range(NC):
        es, ec, gde, tb = per_core[c]
        gsrc = np.zeros(TOT * 128, dtype=np.int16)
        dlv = np.zeros(TOT * 128, dtype=np.float32)
        cfv = np.zeros(TOT * 128, dtype=np.float32)
        off = 0
        for i, (cc, tt) in enumerate(order):
            g = cc * T + tt
            a, b = int(tb[g]), int(tb[g + 1])
            cnt = b - a
            gsrc[off:off + cnt] = es[a:b]
            dlv[off:off + cnt] = (gde[a:b] - g * 128).astype(np.float32)
            cfv[off:off + cnt] = ec[a:b]
            off += CH[i] * 128
        gidx_sb = _wrap16(gsrc, TOT * 8).astype(np.int16)
        dlcf = np.zeros((128, 2 * TOT), dtype=np.float32)
        dlcf[:, :TOT] = dlv.reshape(TOT, 128).T
        dlcf[:, TOT:] = cfv.reshape(TOT, 128).T

        nodes = nodes_of[c]
        pos = newpos[nodes]
        xT = np.zeros((D_IN, n_pad), dtype=BF)
        xT[:, pos] = x[nodes].T.astype(BF)

        sc = np.zeros((128, T), dtype=np.float32)
        sc[pos % 128, pos // 128] = selfc[nodes]

        # local graph slot (0..15) of each node's graph on this core
        slot_of_graph = np.zeros(G, dtype=np.int64)
        slot_of_graph[bins[c]] = np.arange(GP)
        gl = slot_of_graph[batch[nodes]]
        pp = np.zeros((128, T * 16), dtype=np.float32)
        pp[pos % 128, (pos // 128) * 16 + gl] = ginv[batch[nodes]]

        b8 = np.zeros((1, 2 * DH), dtype=BF)
        b8[0, :DH] = (b1 / NC).astype(BF)
        b8[0, DH:] = (b2 / NC).astype(BF)

        in_maps.append({
            "xT": xT,
            "w1": W1.astype(BF),
            "w2": W2.astype(BF),
            "pp": pp.astype(BF),
            "sc": sc,
            "b8": b8,
            "wfc": np.ascontiguousarray(np.broadcast_to(w_fc[:, 0][None, :], (16, DH)).astype(np.float32)),
            "bfc": np.full((16, 1), float(b_fc[0]), dtype=np.float32),
            "gidx": gidx_sb,
            "dlcf": dlcf,
        })

    return in_maps, n_pad, tuple(CH)


# ---------------------------------------------------------------- device build

_CACHE = {}


def build(n_pad, CH):
    key = (n_pad, CH)
    if key in _CACHE:
        return _CACHE[key]
    T = n_pad // 128
    TA = _ta(T)
    TB = T - TA
    TOT = sum(CH)
    GRP = 8  # chunks per dma_gather (1024 idxs; hard per-call limit)

    nc = bacc.Bacc(dynamic_dma_scratch_size=98304)
    xT_in = nc.dram_tensor("xT", [D_IN, n_pad], BF16, kind="ExternalInput")
    w1_in = nc.dram_tensor("w1", [D_IN, DH], BF16, kind="ExternalInput")
    w2_in = nc.dram_tensor("w2", [DH, DH], BF16, kind="ExternalInput")
    pp_in = nc.dram_tensor("pp", [128, T * 16], BF16, kind="ExternalInput")
    sc_in = nc.dram_tensor("sc", [128, T], FP32, kind="ExternalInput")
    b8_in = nc.dram_tensor("b8", [1, 2 * DH], BF16, kind="ExternalInput")
    wfc_in = nc.dram_tensor("wfc", [16, DH], FP32, kind="ExternalInput")
    bfc_in = nc.dram_tensor("bfc", [16, 1], FP32, kind="ExternalInput")
    gidx_in = nc.dram_tensor("gidx", [128, TOT * 8], I16, kind="ExternalInput")
    dlcf_in = nc.dram_tensor("dlcf", [128, 2 * TOT], FP32, kind="ExternalInput")
    out = nc.dram_tensor("out", [16, 1], FP32, kind="ExternalOutput")

    # partial/agg live in partition-major layout: row p of block o holds
    # partition p's features for all tiles of o, so DMA runs are contiguous
    # and the RS shard boundary (dim0 / NC) is exactly one core's block.
    PD = BF16
    zloc = [nc.dram_tensor(f"z{l}loc", [n_pad, DH], BF16) for l in (1, 2)]
    partA = [nc.dram_tensor(f"p{l}A", [NC * 128, TA * DH], PD) for l in (1, 2)]
    partB = [nc.dram_tensor(f"p{l}B", [NC * 128, TB * DH], PD) for l in (1, 2)]
    aggA = [nc.dram_tensor(f"a{l}A", [128, TA * DH], PD) for l in (1, 2)]
    aggB = [nc.dram_tensor(f"a{l}B", [128, TB * DH], PD) for l in (1, 2)]
    zv = [t.rearrange("(t p) f -> p t f", p=128) for t in zloc]
    pvA = [t.rearrange("(o p) (t f) -> o p t f", p=128, f=DH) for t in partA]
    pvB = [t.rearrange("(o p) (t f) -> o p t f", p=128, f=DH) for t in partB]
    avA = [t.rearrange("p (t f) -> p t f", f=DH) for t in aggA]
    avB = [t.rearrange("p (t f) -> p t f", f=DH) for t in aggB]

    with TileContext(nc) as tc:
        with (
            tc.tile_pool(name="const", bufs=1) as const,
            tc.tile_pool(name="gp", bufs=5) as gp,
            tc.tile_pool(name="sp", bufs=6) as sp,
            tc.tile_pool(name="pw", bufs=4) as pw,
            tc.tile_pool(name="ap", bufs=2) as app,
            tc.tile_pool(name="hp", bufs=8) as hp,
            tc.tile_pool(name="smp", bufs=8) as smp,
            tc.tile_pool(name="tp", bufs=4) as tp,
            tc.tile_pool(name="fp", bufs=1) as fp,
            tc.tile_pool(name="psA", bufs=2, space="PSUM") as psA,
            tc.tile_pool(name="psM", bufs=3, space="PSUM") as psM,
            tc.tile_pool(name="psT", bufs=2, space="PSUM") as psT,
            tc.tile_pool(name="psP", bufs=1, space="PSUM") as psP,
        ):
            # ---- constant loads
            xT_sb = const.tile([128, 4, n_pad], BF16)
            for k in range(4):
                nc.sync.dma_start(out=xT_sb[:, k, :], in_=xT_in[k * 128:(k + 1) * 128, :])
            w1_sb = const.tile([128, 4, DH], BF16)
            for k in range(4):
                nc.sync.dma_start(out=w1_sb[:, k, :], in_=w1_in[k * 128:(k + 1) * 128, :])
            w2_sb = const.tile([128, 2, DH], BF16)
            for k in range(2):
                nc.sync.dma_start(out=w2_sb[:, k, :], in_=w2_in[k * 128:(k + 1) * 128, :])
            gidx_sb = const.tile([128, TOT * 8], I16)
            nc.sync.dma_start(out=gidx_sb[:], in_=gidx_in[:])
            dlcf_sb = const.tile([128, 2 * TOT], FP32)
            nc.sync.dma_start(out=dlcf_sb[:], in_=dlcf_in[:])
            pp_sb = const.tile([128, T * 16], BF16)
            nc.sync.dma_start(out=pp_sb[:], in_=pp_in[:])
            sc_sb = const.tile([128, T], FP32)
            nc.sync.dma_start(out=sc_sb[:], in_=sc_in[:])
            b8_sb = const.tile([1, 2 * DH], BF16)
            nc.sync.dma_start(out=b8_sb[:], in_=b8_in[:])
            wfc_sb = const.tile([16, DH], FP32)
            nc.sync.dma_start(out=wfc_sb[:], in_=wfc_in[:])
            bfc_sb = const.tile([16, 1], FP32)
            nc.sync.dma_start(out=bfc_sb[:], in_=bfc_in[:])

            ident = const.tile([128, 128], BF16)
            make_identity(nc, ident[:])
            ones_sb = const.tile([1, 128], BF16)
            nc.vector.memset(ones_sb[:], 1.0)
            iota_i = const.tile([128, 128], I32)
            nc.gpsimd.iota(iota_i[:], pattern=[[1, 128]], base=0, channel_multiplier=0)
            iota_f = const.tile([128, 128], BF16)
            nc.vector.tensor_copy(iota_f[:], iota_i[:])

            # persistent z (bf16) per layer, also staging for DRAM writes
            z1sb = const.tile([128, T, DH], BF16)
            z2sb = const.tile([128, T, DH], BF16)
            zsb = [z1sb, z2sb]

            # ---- phase A: z1 = x @ W1 (own nodes)
            for t in range(T):
                acc = psA.tile([128, DH], FP32, space="PSUM", tag="psA")
                for k in range(4):
                    nc.tensor.matmul(
                        out=acc[:], lhsT=xT_sb[:, k, t * 128:(t + 1) * 128],
                        rhs=w1_sb[:, k, :], start=(k == 0), stop=(k == 3))
                nc.vector.tensor_copy(zsb[0][:, t, :], acc[:])
            for w in range(0, T, 8):
                e = min(T, w + 8)
                nc.sync.dma_start(out=zv[0][:, w:e, :], in_=zsb[0][:, w:e, :])

            # ---- two GCN layers
            pool_holder = [None]
            for l in range(2):
                gtile = [None, None]  # (group id, tile)

                def get_msg(q, l=l, gtile=gtile):
                    grp = q // GRP
                    if gtile[0] != grp:
                        sz = min(GRP, TOT - grp * GRP)
                        gt = gp.tile([128, sz, DH], BF16, tag="g")
                        nc.gpsimd.dma_gather(
                            out_ap=gt[:],
                            in_ap=zloc[l][:],
                            idxs_ap=gidx_sb[:, grp * GRP * 8:(grp * GRP + sz) * 8],
                            num_idxs=sz * 128,
                            num_idxs_reg=sz * 128,
                            elem_size=DH,
                        )
                        gtile[0], gtile[1] = grp, gt
                    return gtile[1][:, q % GRP, :]

                q0 = 0   # global chunk counter (layer-local)
                i0 = 0   # processing tile counter

                def scatter_phase(PT, pview, q0, i0, l=l, get_msg=get_msg,
                                  emit_mid=None, mid_at=8):
                    # processing order: dst block o major, local tile t minor;
                    # pstage groups of 4 tiles never cross a block boundary
                    pstage = None
                    off = q0
                    for i in range(NC * PT):
                        if emit_mid is not None and i == mid_at:
                            emit_mid()
                        o, t = i // PT, i % PT
                        ch = CH[i0 + i]
                        acc = psM.tile([128, DH], FP32, space="PSUM", tag="psM")
                        for j in range(ch):
                            q = off + j
                            msg = get_msg(q)
                            S = sp.tile([128, 128], BF16, tag="S")
                            nc.vector.tensor_scalar(
                                out=S[:], in0=iota_f[:],
                                scalar1=dlcf_sb[:, q:q + 1],
                                scalar2=dlcf_sb[:, TOT + q:TOT + q + 1],
                                op0=mybir.AluOpType.is_equal,
                                op1=mybir.AluOpType.mult)
                            nc.tensor.matmul(out=acc[:], lhsT=S[:], rhs=msg,
                                             start=(j == 0), stop=False)
                        nc.tensor.matmul(
                            out=acc[:], lhsT=ones_sb[:],
                            rhs=b8_sb[0:1, l * DH:(l + 1) * DH],
                            start=False, stop=True)
                        if t % 4 == 0:
                            pstage = pw.tile([128, 4, DH], PD, tag="pst")
                        nc.scalar.activation(out=pstage[:, t % 4, :], in_=acc[:],
                                             func=mybir.ActivationFunctionType.Copy)
                        if t % 4 == 3 or t == PT - 1:
                            w0 = (t // 4) * 4
                            nc.sync.dma_start(out=pview[o, :, w0:t + 1, :],
                                              in_=pstage[:, :t + 1 - w0, :])
                        off += ch
                    return off

                def post_phase(t_lo, t_hi, aview, l=l):
                    # engine-stage order inside each group so tiles pipeline
                    # instead of serializing on cross-engine semaphores
                    for w in range(t_lo, t_hi, 8):
                        e = min(t_hi, w + 8)
                        ast = app.tile([128, 8, DH], PD, tag="agg")
                        nc.sync.dma_start(out=ast[:, :e - w, :],
                                          in_=aview[:, w - t_lo:e - t_lo, :])
                        sms = []
                        for t in range(w, e):
                            tmp = tp.tile([128, DH], BF16, tag="tmp")
                            nc.vector.tensor_scalar_mul(
                                tmp[:], zsb[l][:, t, :], sc_sb[:, t:t + 1])
                            sm = smp.tile([128, DH], BF16, tag="sm")
                            nc.vector.tensor_tensor(out=sm[:], in0=tmp[:],
                                                    in1=ast[:, t - w, :],
                                                    op=mybir.AluOpType.add)
                            sms.append(sm)
                        hs = []
                        for t in range(w, e):
                            h = hp.tile([128, DH], BF16, tag="h")
                            nc.scalar.activation(out=h[:], in_=sms[t - w][:],
                                                 func=mybir.ActivationFunctionType.Relu)
                            hs.append(h)
                        for t in range(w, e):
                            h = hs[t - w]
                            if l == 0:
                                hT = []
                                for half in range(2):
                                    pt = psT.tile([128, 128], BF16, space="PSUM", tag="psT")
                                    nc.tensor.transpose(
                                        out=pt[:], in_=h[:, half * 128:(half + 1) * 128],
                                        identity=ident[:])
                                    ht = tp.tile([128, 128], BF16, tag="hT")
                                    nc.vector.tensor_copy(ht[:], pt[:])
                                    hT.append(ht)
                                accz = psA.tile([128, DH], FP32, space="PSUM", tag="psA")
                                for half in range(2):
                                    nc.tensor.matmul(out=accz[:], lhsT=hT[half][:],
                                                     rhs=w2_sb[:, half, :],
                                                     start=(half == 0), stop=(half == 1))
                                nc.vector.tensor_copy(zsb[1][:, t, :], accz[:])
                            else:
                                if t == 0:
                                    pool_holder[0] = psP.tile([16, DH], FP32, space="PSUM",
                                                              tag="psP", name="pool_acc")
                                pool_acc = pool_holder[0]
                                nc.tensor.matmul(out=pool_acc[:],
                                                 lhsT=pp_sb[:, t * 16:(t + 1) * 16],
                                                 rhs=h[:], start=(t == 0), stop=(t == T - 1),
                                                 skip_group_check=True)
                        if l == 0:
                            nc.sync.dma_start(out=zv[1][:, w:e, :],
                                              in_=zsb[1][:, w:e, :])

                # phase A scatter; RS-A is emitted a few tiles into the B
                # stream so its sequencer wait doesn't stall the B gathers
                q0 = scatter_phase(TA, pvA[l], q0, 0)

                def emit_rsA(l=l):
                    nc.gpsimd.collective_compute(
                        "ReduceScatter", mybir.AluOpType.add,
                        ins=[partA[l][:]], outs=[aggA[l][:]],
                        replica_groups=[list(range(NC))])

                # phase B scatter + RS-B
                q0 = scatter_phase(TB, pvB[l], q0, NC * TA,
                                   emit_mid=emit_rsA, mid_at=4)
                nc.gpsimd.collective_compute(
                    "ReduceScatter", mybir.AluOpType.add,
                    ins=[partB[l][:]], outs=[aggB[l][:]],
                    replica_groups=[list(range(NC))])
                # post-RS work (A overlaps RS-B). The fence keeps the agg
                # loads (which wait on the collectives) from being scheduled
                # ahead of the B-phase partial writes on the SP queue.
                tc.no_sync_barrier()
                post_phase(0, TA, avA[l])
                post_phase(TA, T, avB[l])

            # ---- fc head: out = pooled @ w_fc + b_fc
            pooled = fp.tile([16, DH], FP32)
            nc.vector.tensor_copy(pooled[:], pool_holder[0][:])
            prod = fp.tile([16, DH], FP32)
            nc.vector.tensor_tensor(out=prod[:], in0=pooled[:], in1=wfc_sb[:],
                                    op=mybir.AluOpType.mult)
            red = fp.tile([16, 1], FP32)
            nc.vector.reduce_sum(red[:], prod[:], axis=mybir.AxisListType.X)
            outv = fp.tile([16, 1], FP32)
            nc.vector.tensor_scalar_add(outv[:], red[:], bfc_sb[:])
            nc.sync.dma_start(out=out[:], in_=outv[:])

    nc.finalize()
    _CACHE[key] = nc
    return nc


# ---------------------------------------------------------------- entry points

def _run(inputs, trace=False):
    in_maps, n_pad, CH = prep(**inputs)
    nc = build(n_pad, CH)
    r = run_bass_kernel_spmd(nc, in_maps, list(range(NC)), trace=trace)
    batch = np.asarray(inputs["batch"]).astype(np.int64)
    bins = _assign_graphs(np.bincount(batch, minlength=G).astype(np.int64))
    out = np.zeros(G, dtype=np.float32)
    for c in range(NC):
        out[bins[c]] = r.results[c]["out"][:, 0]
    return out, r


def kernel(**inputs):
    out, _ = _run(inputs, trace=False)
    return out


def kernel_traced(**inputs):
    out, r = _run(inputs, trace=True)
    return out, r


# revision 64
# speedup vs baseline: 1.0025x; 1.0025x over previous
"""DeepfakeGNN (2x GCNConv + mean-pool + fc) on 8 Trainium2 NeuronCores.

Scatter/ReduceScatter dataflow: graphs are split 16-per-core (batch is sorted,
so each core owns a contiguous node range). Edges are assigned to the core
owning their SRC node. Per layer: each core computes z = H @ W for its own
nodes (bf16), scatters edge messages into full-size partial accumulators via
dma_gather (rows by local src index) + one-hot segment matmuls on the tensor
engine, then ReduceScatter(add) returns each core the summed aggregation for
its own rows. The scatter/RS is split into two phases (A = first TA local
tiles of every core, B = the rest) so RS-A overlaps the B scatter and the
post-RS work of A overlaps RS-B. Bias is accumulated in-scatter as a closing
ones x (bias/8) matmul; self-loops are applied post-RS as a per-row scaled
copy of local z. Pool + fc are local; the host concatenates the per-core [16]
outputs.

Self-contained: only numpy + ml_dtypes + concourse (preinstalled).
"""
import numpy as np
import ml_dtypes

import concourse.mybir as mybir
from concourse import bacc
from concourse.bass_utils import run_bass_kernel_spmd
from concourse.masks import make_identity
from concourse.tile import TileContext

NC = 8          # cores
N = 20000       # nodes
D_IN = 512
DH = 256
G = 128         # graphs
GP = G // NC    # graphs per core

FP32 = mybir.dt.float32
BF16 = mybir.dt.bfloat16
I16 = mybir.dt.int16
I32 = mybir.dt.int32

BF = ml_dtypes.bfloat16


def _ta(T):
    """A-phase size: RS-A must hide under the B scatter."""
    return max(1, min(T - 1, (T + 1) // 2 + 1))


def _assign_graphs(gcnt):
    """Assign 16 graphs per core, balancing total node count (greedy LPT).
    Returns [NC][GP] graph ids."""
    order = np.argsort(-gcnt, kind="stable")
    totals = np.zeros(NC, dtype=np.int64)
    counts = np.zeros(NC, dtype=np.int64)
    bins = [[] for _ in range(NC)]
    for g in order:
        c = min((c for c in range(NC) if counts[c] < GP),
                key=lambda c: totals[c])
        bins[c].append(int(g))
        totals[c] += gcnt[g]
        counts[c] += 1
    return bins


def _pack_tiles(vecs, T):
    """Greedy-pack nodes (rows of vecs [n, NC] = per-src-core in-edge counts)
    into T tiles of <=128 nodes, minimizing the max per-core count per tile
    (target <= 256 so every tile needs only 2 chunks). Returns pos[n]."""
    n = vecs.shape[0]
    order = np.argsort(-vecs.sum(axis=1), kind="stable")
    sums = np.zeros((T, NC), dtype=np.int64)
    cnts = np.zeros(T, dtype=np.int64)
    pos = np.zeros(n, dtype=np.int64)
    for i in order:
        v = vecs[i]
        score = np.max(sums + v[None, :], axis=1).astype(np.float64)
        score[cnts >= 128] = np.inf
        t = int(np.argmin(score))
        pos[i] = t * 128 + cnts[t]
        sums[t] += v
        cnts[t] += 1
    return pos


# ---------------------------------------------------------------- host prep

def _wrap16(arr, cols):
    """Lay out a flat int array [cols*16] -> [128, cols] in dma_gather idx
    order (idx j at [j%16, j//16], replicated across the 8 q7 cores)."""
    a = arr.reshape(cols, 16).T  # [16, cols]
    return np.ascontiguousarray(np.tile(a, (8, 1)))


def prep(x, edge_index, batch, W1, b1, W2, b2, w_fc, b_fc):
    x = np.asarray(x, dtype=np.float32)
    ei = np.asarray(edge_index).astype(np.int64)
    batch = np.asarray(batch).astype(np.int64)
    W1 = np.asarray(W1, dtype=np.float32)
    b1 = np.asarray(b1, dtype=np.float32)
    W2 = np.asarray(W2, dtype=np.float32)
    b2 = np.asarray(b2, dtype=np.float32)
    w_fc = np.asarray(w_fc, dtype=np.float32)
    b_fc = np.asarray(b_fc, dtype=np.float32)

    n = x.shape[0]
    src, dst = ei[0], ei[1]

    # degree includes self-loop (reference concatenates loops)
    deg = (np.bincount(dst, minlength=n) + 1.0).astype(np.float32)
    dinv = (1.0 / np.sqrt(deg, dtype=np.float32)).astype(np.float32)
    coef = (dinv[src] * dinv[dst]).astype(np.float32)
    selfc = (dinv * dinv).astype(np.float32)

    # graphs -> cores balanced by node count; nodes follow their graph
    gcnt = np.bincount(batch, minlength=G).astype(np.int64)
    bins = _assign_graphs(gcnt)
    core_of_graph = np.zeros(G, dtype=np.int64)
    for c in range(NC):
        core_of_graph[bins[c]] = c
    owner = core_of_graph[batch]                  # owning core per node
    nodes_of = [np.where(owner == c)[0] for c in range(NC)]
    n_c = np.array([len(v) for v in nodes_of])
    # one extra tile of headroom so the packer can keep every tile at 2 chunks
    n_pad = int((int(np.ceil(n_c.max() / 128.0)) + 1) * 128)
    T = n_pad // 128
    TA = _ta(T)

    # Permute nodes within each core's block so that each 128-row dst tile
    # draws <=256 in-edges from every src core (2 chunks per tile).
    vec = np.zeros((n, NC), dtype=np.int64)
    np.add.at(vec, (dst, owner[src]), 1)
    newpos = np.zeros(n, dtype=np.int64)
    for c in range(NC):
        newpos[nodes_of[c]] = _pack_tiles(vec[nodes_of[c]], T)

    gd = owner * n_pad + newpos                   # global padded slot

    # processing order: phase A tiles (t < TA, c-major), then phase B
    order = [(c, t) for rng in (range(TA), range(TA, T))
             for c in range(NC) for t in rng]

    # per-core edge streams (src owned by core), grouped by global dst tile
    per_core = []
    for c in range(NC):
        m = owner[src] == c
        es = newpos[src[m]]                          # local z row (permuted)
        gde = gd[dst[m]]
        ec = coef[m]
        o = np.argsort(gde, kind="stable")
        es, ec, gde = es[o], ec[o], gde[o]
        tb = np.searchsorted(gde, np.arange(0, NC * n_pad + 1, 128))
        per_core.append((es, ec, gde, tb))

    # shared chunk schedule in processing order
    CH = []
    for (c_t) in order:
        cc, tt = c_t
        g = cc * T + tt
        mx = 1
        for c in range(NC):
            tb = per_core[c][3]
            cnt = int(tb[g + 1] - tb[g])
            mx = max(mx, (cnt + 127) // 128)
        CH.append(mx)
    TOT = sum(CH)

    # per-graph 1/count for mean pooling
    gcnt = np.bincount(batch, minlength=G).astype(np.float32)
    ginv = 1.0 / np.maximum(gcnt, 1.0)

    in_maps = []
    for c in range(NC):
        es, ec, gde, tb = per_core[c]
        gsrc = np.zeros(TOT * 128, dtype=np.int16)
        dlv = np.zeros(TOT * 128, dtype=np.float32)
        cfv = np.zeros(TOT * 128, dtype=np.float32)
        off = 0
        for i, (cc, tt) in enumerate(order):
            g = cc * T + tt
            a, b = int(tb[g]), int(tb[g + 1])
            cnt = b - a
            gsrc[off:off + cnt] = es[a:b]
            dlv[off:off + cnt] = (gde[a:b] - g * 128).astype(np.float32)
            cfv[off:off + cnt] = ec[a:b]
            off += CH[i] * 128
        gidx_sb = _wrap16(gsrc, TOT * 8).astype(np.int16)
        dlcf = np.zeros((128, 2 * TOT), dtype=np.float32)
        dlcf[:, :TOT] = dlv.reshape(TOT, 128).T
        dlcf[:, TOT:] = cfv.reshape(TOT, 128).T

        nodes = nodes_of[c]
        pos = newpos[nodes]
        xT = np.zeros((D_IN, n_pad), dtype=BF)
        xT[:, pos] = x[nodes].T.astype(BF)

        sc = np.zeros((128, T), dtype=np.float32)
        sc[pos % 128, pos // 128] = selfc[nodes]

        # local graph slot (0..15) of each node's graph on this core
        slot_of_graph = np.zeros(G, dtype=np.int64)
        slot_of_graph[bins[c]] = np.arange(GP)
        gl = slot_of_graph[batch[nodes]]
        pp = np.zeros((128, T * 16), dtype=np.float32)
        pp[pos % 128, (pos // 128) * 16 + gl] = ginv[batch[nodes]]

        b8 = np.zeros((1, 2 * DH), dtype=BF)
        b8[0, :DH] = (b1 / NC).astype(BF)
        b8[0, DH:] = (b2 / NC).astype(BF)

        in_maps.append({
            "xT": xT,
            "w1": W1.astype(BF),
            "w2": W2.astype(BF),
            "pp": pp.astype(BF),
            "sc": sc,
            "b8": b8,
            "wfc": np.ascontiguousarray(np.broadcast_to(w_fc[:, 0][None, :], (16, DH)).astype(np.float32)),
            "bfc": np.full((16, 1), float(b_fc[0]), dtype=np.float32),
            "gidx": gidx_sb,
            "dlcf": dlcf,
        })

    return in_maps, n_pad, tuple(CH)


# ---------------------------------------------------------------- device build

_CACHE = {}


def build(n_pad, CH):
    key = (n_pad, CH)
    if key in _CACHE:
        return _CACHE[key]
    T = n_pad // 128
    TA = _ta(T)
    TB = T - TA
    TOT = sum(CH)
    GRP = 8  # chunks per dma_gather (1024 idxs; hard per-call limit)

    nc = bacc.Bacc(dynamic_dma_scratch_size=98304)
    xT_in = nc.dram_tensor("xT", [D_IN, n_pad], BF16, kind="ExternalInput")
    w1_in = nc.dram_tensor("w1", [D_IN, DH], BF16, kind="ExternalInput")
    w2_in = nc.dram_tensor("w2", [DH, DH], BF16, kind="ExternalInput")
    pp_in = nc.dram_tensor("pp", [128, T * 16], BF16, kind="ExternalInput")
    sc_in = nc.dram_tensor("sc", [128, T], FP32, kind="ExternalInput")
    b8_in = nc.dram_tensor("b8", [1, 2 * DH], BF16, kind="ExternalInput")
    wfc_in = nc.dram_tensor("wfc", [16, DH], FP32, kind="ExternalInput")
    bfc_in = nc.dram_tensor("bfc", [16, 1], FP32, kind="ExternalInput")
    gidx_in = nc.dram_tensor("gidx", [128, TOT * 8], I16, kind="ExternalInput")
    dlcf_in = nc.dram_tensor("dlcf", [128, 2 * TOT], FP32, kind="ExternalInput")
    out = nc.dram_tensor("out", [16, 1], FP32, kind="ExternalOutput")

    # partial/agg live in partition-major layout: row p of block o holds
    # partition p's features for all tiles of o, so DMA runs are contiguous
    # and the RS shard boundary (dim0 / NC) is exactly one core's block.
    PD = BF16
    zloc = [nc.dram_tensor(f"z{l}loc", [n_pad, DH], BF16) for l in (1, 2)]
    partA = [nc.dram_tensor(f"p{l}A", [NC * 128, TA * DH], PD) for l in (1, 2)]
    partB = [nc.dram_tensor(f"p{l}B", [NC * 128, TB * DH], PD) for l in (1, 2)]
    aggA = [nc.dram_tensor(f"a{l}A", [128, TA * DH], PD) for l in (1, 2)]
    aggB = [nc.dram_tensor(f"a{l}B", [128, TB * DH], PD) for l in (1, 2)]
    zv = [t.rearrange("(t p) f -> p t f", p=128) for t in zloc]
    pvA = [t.rearrange("(o p) (t f) -> o p t f", p=128, f=DH) for t in partA]
    pvB = [t.rearrange("(o p) (t f) -> o p t f", p=128, f=DH) for t in partB]
    avA = [t.rearrange("p (t f) -> p t f", f=DH) for t in aggA]
    avB = [t.rearrange("p (t f) -> p t f", f=DH) for t in aggB]

    with TileContext(nc) as tc:
        with (
            tc.tile_pool(name="const", bufs=1) as const,
            tc.tile_pool(name="gp", bufs=5) as gp,
            tc.tile_pool(name="sp", bufs=6) as sp,
            tc.tile_pool(name="pw", bufs=4) as pw,
            tc.tile_pool(name="ap", bufs=2) as app,
            tc.tile_pool(name="hp", bufs=8) as hp,
            tc.tile_pool(name="smp", bufs=8) as smp,
            tc.tile_pool(name="tp", bufs=4) as tp,
            tc.tile_pool(name="fp", bufs=1) as fp,
            tc.tile_pool(name="psA", bufs=2, space="PSUM") as psA,
            tc.tile_pool(name="psM", bufs=3, space="PSUM") as psM,
            tc.tile_pool(name="psT", bufs=2, space="PSUM") as psT,
            tc.tile_pool(name="psP", bufs=1, space="PSUM") as psP,
        ):
            # ---- constant loads
            xT_sb = const.tile([128, 4, n_pad], BF16)
            for k in range(4):
                nc.sync.dma_start(out=xT_sb[:, k, :], in_=xT_in[k * 128:(k + 1) * 128, :])
            w1_sb = const.tile([128, 4, DH], BF16)
            for k in range(4):
                nc.sync.dma_start(out=w1_sb[:, k, :], in_=w1_in[k * 128:(k + 1) * 128, :])
            w2_sb = const.tile([128, 2, DH], BF16)
            for k in range(2):
                nc.sync.dma_start(out=w2_sb[:, k, :], in_=w2_in[k * 128:(k + 1) * 128, :])
            gidx_sb = const.tile([128, TOT * 8], I16)
            nc.sync.dma_start(out=gidx_sb[:], in_=gidx_in[:])
            dlcf_sb = const.tile([128, 2 * TOT], FP32)
            nc.sync.dma_start(out=dlcf_sb[:], in_=dlcf_in[:])
            pp_sb = const.tile([128, T * 16], BF16)
            nc.sync.dma_start(out=pp_sb[:], in_=pp_in[:])
            sc_sb = const.tile([128, T], FP32)
            nc.sync.dma_start(out=sc_sb[:], in_=sc_in[:])
            b8_sb = const.tile([1, 2 * DH], BF16)
            nc.sync.dma_start(out=b8_sb[:], in_=b8_in[:])
            wfc_sb = const.tile([16, DH], FP32)
            nc.sync.dma_start(out=wfc_sb[:], in_=wfc_in[:])
            bfc_sb = const.tile([16, 1], FP32)
            nc.sync.dma_start(out=bfc_sb[:], in_=bfc_in[:])

            ident = const.tile([128, 128], BF16)
            make_identity(nc, ident[:])
            ones_sb = const.tile([1, 128], BF16)
            nc.vector.memset(ones_sb[:], 1.0)
            iota_i = const.tile([128, 128], I32)
            nc.gpsimd.iota(iota_i[:], pattern=[[1, 128]], base=0, channel_multiplier=0)
            iota_f = const.tile([128, 128], BF16)
            nc.vector.tensor_copy(iota_f[:], iota_i[:])

            # persistent z (bf16) per layer, also staging for DRAM writes
            z1sb = const.tile([128, T, DH], BF16)
            z2sb = const.tile([128, T, DH], BF16)
            zsb = [z1sb, z2sb]

            # ---- phase A: z1 = x @ W1 (own nodes)
            for t in range(T):
                acc = psA.tile([128, DH], FP32, space="PSUM", tag="psA")
                for k in range(4):
                    nc.tensor.matmul(
                        out=acc[:], lhsT=xT_sb[:, k, t * 128:(t + 1) * 128],
                        rhs=w1_sb[:, k, :], start=(k == 0), stop=(k == 3))
                nc.vector.tensor_copy(zsb[0][:, t, :], acc[:])
            for w in range(0, T, 8):
                e = min(T, w + 8)
                nc.sync.dma_start(out=zv[0][:, w:e, :], in_=zsb[0][:, w:e, :])
            # pre-scale z by self-loop coef in place (post-RS adds it directly)
            for t in range(T):
                nc.vector.tensor_scalar_mul(
                    zsb[0][:, t, :], zsb[0][:, t, :], sc_sb[:, t:t + 1])

            # ---- two GCN layers
            pool_holder = [None]
            for l in range(2):
                gtile = [None, None]  # (group id, tile)

                def get_msg(q, l=l, gtile=gtile):
                    grp = q // GRP
                    if gtile[0] != grp:
                        sz = min(GRP, TOT - grp * GRP)
                        gt = gp.tile([128, sz, DH], BF16, tag="g")
                        nc.gpsimd.dma_gather(
                            out_ap=gt[:],
                            in_ap=zloc[l][:],
                            idxs_ap=gidx_sb[:, grp * GRP * 8:(grp * GRP + sz) * 8],
                            num_idxs=sz * 128,
                            num_idxs_reg=sz * 128,
                            elem_size=DH,
                        )
                        gtile[0], gtile[1] = grp, gt
                    return gtile[1][:, q % GRP, :]

                q0 = 0   # global chunk counter (layer-local)
                i0 = 0   # processing tile counter

                def scatter_phase(PT, pview, q0, i0, l=l, get_msg=get_msg,
                                  emit_mid=None, mid_at=8):
                    # processing order: dst block o major, local tile t minor;
                    # pstage groups of 4 tiles never cross a block boundary
                    pstage = None
                    off = q0
                    for i in range(NC * PT):
                        if emit_mid is not None and i == mid_at:
                            emit_mid()
                        o, t = i // PT, i % PT
                        ch = CH[i0 + i]
                        acc = psM.tile([128, DH], FP32, space="PSUM", tag="psM")
                        for j in range(ch):
                            q = off + j
                            msg = get_msg(q)
                            S = sp.tile([128, 128], BF16, tag="S")
                            nc.vector.tensor_scalar(
                                out=S[:], in0=iota_f[:],
                                scalar1=dlcf_sb[:, q:q + 1],
                                scalar2=dlcf_sb[:, TOT + q:TOT + q + 1],
                                op0=mybir.AluOpType.is_equal,
                                op1=mybir.AluOpType.mult)
                            nc.tensor.matmul(out=acc[:], lhsT=S[:], rhs=msg,
                                             start=(j == 0), stop=False)
                        nc.tensor.matmul(
                            out=acc[:], lhsT=ones_sb[:],
                            rhs=b8_sb[0:1, l * DH:(l + 1) * DH],
                            start=False, stop=True)
                        if t % 4 == 0:
                            pstage = pw.tile([128, 4, DH], PD, tag="pst")
                        nc.scalar.activation(out=pstage[:, t % 4, :], in_=acc[:],
                                             func=mybir.ActivationFunctionType.Copy)
                        if t % 4 == 3 or t == PT - 1:
                            w0 = (t // 4) * 4
                            nc.sync.dma_start(out=pview[o, :, w0:t + 1, :],
                                              in_=pstage[:, :t + 1 - w0, :])
                        off += ch
                    return off

                def post_phase(t_lo, t_hi, aview, l=l):
                    # engine-stage order inside each group so tiles pipeline
                    # instead of serializing on cross-engine semaphores
                    for w in range(t_lo, t_hi, 8):
                        e = min(t_hi, w + 8)
                        ast = app.tile([128, 8, DH], PD, tag="agg")
                        nc.sync.dma_start(out=ast[:, :e - w, :],
                                          in_=aview[:, w - t_lo:e - t_lo, :])
                        hs = []
                        for t in range(w, e):
                            sm = smp.tile([128, DH], BF16, tag="sm")
                            nc.vector.tensor_tensor(out=sm[:], in0=zsb[l][:, t, :],
                                                    in1=ast[:, t - w, :],
                                                    op=mybir.AluOpType.add)
                            h = hp.tile([128, DH], BF16, tag="h")
                            nc.vector.tensor_scalar_max(h[:], sm[:], 0.0)
                            hs.append(h)
                        for t in range(w, e):
                            h = hs[t - w]
                            if l == 0:
                                hT = []
                                for half in range(2):
                                    pt = psT.tile([128, 128], BF16, space="PSUM", tag="psT")
                                    nc.tensor.transpose(
                                        out=pt[:], in_=h[:, half * 128:(half + 1) * 128],
                                        identity=ident[:])
                                    ht = tp.tile([128, 128], BF16, tag="hT")
                                    nc.vector.tensor_copy(ht[:], pt[:])
                                    hT.append(ht)
                                accz = psA.tile([128, DH], FP32, space="PSUM", tag="psA")
                                for half in range(2):
                                    nc.tensor.matmul(out=accz[:], lhsT=hT[half][:],
                                                     rhs=w2_sb[:, half, :],
                                                     start=(half == 0), stop=(half == 1))
                                nc.vector.tensor_copy(zsb[1][:, t, :], accz[:])
                            else:
                                if t == 0:
                                    pool_holder[0] = psP.tile([16, DH], FP32, space="PSUM",
                                                              tag="psP", name="pool_acc")
                                pool_acc = pool_holder[0]
                                nc.tensor.matmul(out=pool_acc[:],
                                                 lhsT=pp_sb[:, t * 16:(t + 1) * 16],
                                                 rhs=h[:], start=(t == 0), stop=(t == T - 1),
                                                 skip_group_check=True)
                        if l == 0:
                            nc.sync.dma_start(out=zv[1][:, w:e, :],
                                              in_=zsb[1][:, w:e, :])

                # phase A scatter; RS-A is emitted a few tiles into the B
                # stream so its sequencer wait doesn't stall the B gathers
                q0 = scatter_phase(TA, pvA[l], q0, 0)

                def emit_rsA(l=l):
                    nc.gpsimd.collective_compute(
                        "ReduceScatter", mybir.AluOpType.add,
                        ins=[partA[l][:]], outs=[aggA[l][:]],
                        replica_groups=[list(range(NC))])

                # phase B scatter + RS-B
                q0 = scatter_phase(TB, pvB[l], q0, NC * TA,
                                   emit_mid=emit_rsA, mid_at=4)
                nc.gpsimd.collective_compute(
                    "ReduceScatter", mybir.AluOpType.add,
                    ins=[partB[l][:]], outs=[aggB[l][:]],
                    replica_groups=[list(range(NC))])
                # post-RS work (A overlaps RS-B). The fence keeps the agg
                # loads (which wait on the collectives) from being scheduled
                # ahead of the B-phase partial writes on the SP queue.
                tc.no_sync_barrier()
                post_phase(0, TA, avA[l])
                post_phase(TA, T, avB[l])
                if l == 0:
                    # pre-scale z2 by self coef in place (off the critical
                    # path; runs during the layer-2 scatter ramp)
                    for t in range(T):
                        nc.vector.tensor_scalar_mul(
                            zsb[1][:, t, :], zsb[1][:, t, :], sc_sb[:, t:t + 1])

            # ---- fc head: out = pooled @ w_fc + b_fc
            pooled = fp.tile([16, DH], FP32)
            nc.vector.tensor_copy(pooled[:], pool_holder[0][:])
            prod = fp.tile([16, DH], FP32)
            nc.vector.tensor_tensor(out=prod[:], in0=pooled[:], in1=wfc_sb[:],
                                    op=mybir.AluOpType.mult)
            red = fp.tile([16, 1], FP32)
            nc.vector.reduce_sum(red[:], prod[:], axis=mybir.AxisListType.X)
            outv = fp.tile([16, 1], FP32)
            nc.vector.tensor_scalar_add(outv[:], red[:], bfc_sb[:])
            nc.sync.dma_start(out=out[:], in_=outv[:])

    nc.finalize()
    _CACHE[key] = nc
    return nc


# ---------------------------------------------------------------- entry points

def _run(inputs, trace=False):
    in_maps, n_pad, CH = prep(**inputs)
    nc = build(n_pad, CH)
    r = run_bass_kernel_spmd(nc, in_maps, list(range(NC)), trace=trace)
    batch = np.asarray(inputs["batch"]).astype(np.int64)
    bins = _assign_graphs(np.bincount(batch, minlength=G).astype(np.int64))
    out = np.zeros(G, dtype=np.float32)
    for c in range(NC):
        out[bins[c]] = r.results[c]["out"][:, 0]
    return out, r


def kernel(**inputs):
    out, _ = _run(inputs, trace=False)
    return out


def kernel_traced(**inputs):
    out, r = _run(inputs, trace=True)
    return out, r


# revision 68
# speedup vs baseline: 1.0173x; 1.0148x over previous
"""DeepfakeGNN (2x GCNConv + mean-pool + fc) on 8 Trainium2 NeuronCores.

Scatter/ReduceScatter dataflow: graphs are split 16-per-core (batch is sorted,
so each core owns a contiguous node range). Edges are assigned to the core
owning their SRC node. Per layer: each core computes z = H @ W for its own
nodes (bf16), scatters edge messages into full-size partial accumulators via
dma_gather (rows by local src index) + one-hot segment matmuls on the tensor
engine, then ReduceScatter(add) returns each core the summed aggregation for
its own rows. The scatter/RS is split into two phases (A = first TA local
tiles of every core, B = the rest) so RS-A overlaps the B scatter and the
post-RS work of A overlaps RS-B. Bias is accumulated in-scatter as a closing
ones x (bias/8) matmul; self-loops are applied post-RS as a per-row scaled
copy of local z. Pool + fc are local; the host concatenates the per-core [16]
outputs.

Self-contained: only numpy + ml_dtypes + concourse (preinstalled).
"""
import numpy as np
import ml_dtypes

import concourse.mybir as mybir
from concourse import bacc
from concourse.bass_utils import run_bass_kernel_spmd
from concourse.masks import make_identity
from concourse.tile import TileContext

NC = 8          # cores
N = 20000       # nodes
D_IN = 512
DH = 256
G = 128         # graphs
GP = G // NC    # graphs per core

FP32 = mybir.dt.float32
BF16 = mybir.dt.bfloat16
I16 = mybir.dt.int16
I32 = mybir.dt.int32

BF = ml_dtypes.bfloat16


def _ta(T):
    """A-phase size: RS-A must hide under the B scatter (~5/8 empirically)."""
    return max(1, min(T - 1, (5 * T) // 8))


def _assign_graphs(gcnt):
    """Assign 16 graphs per core, balancing total node count (greedy LPT).
    Returns [NC][GP] graph ids."""
    order = np.argsort(-gcnt, kind="stable")
    totals = np.zeros(NC, dtype=np.int64)
    counts = np.zeros(NC, dtype=np.int64)
    bins = [[] for _ in range(NC)]
    for g in order:
        c = min((c for c in range(NC) if counts[c] < GP),
                key=lambda c: totals[c])
        bins[c].append(int(g))
        totals[c] += gcnt[g]
        counts[c] += 1
    return bins


def _pack_tiles(vecs, T):
    """Greedy-pack nodes (rows of vecs [n, NC] = per-src-core in-edge counts)
    into T tiles of <=128 nodes, minimizing the max per-core count per tile
    (target <= 256 so every tile needs only 2 chunks). Returns pos[n]."""
    n = vecs.shape[0]
    order = np.argsort(-vecs.sum(axis=1), kind="stable")
    sums = np.zeros((T, NC), dtype=np.int64)
    cnts = np.zeros(T, dtype=np.int64)
    pos = np.zeros(n, dtype=np.int64)
    for i in order:
        v = vecs[i]
        score = np.max(sums + v[None, :], axis=1).astype(np.float64)
        score[cnts >= 128] = np.inf
        t = int(np.argmin(score))
        pos[i] = t * 128 + cnts[t]
        sums[t] += v
        cnts[t] += 1
    return pos


# ---------------------------------------------------------------- host prep

def _wrap16(arr, cols):
    """Lay out a flat int array [cols*16] -> [128, cols] in dma_gather idx
    order (idx j at [j%16, j//16], replicated across the 8 q7 cores)."""
    a = arr.reshape(cols, 16).T  # [16, cols]
    return np.ascontiguousarray(np.tile(a, (8, 1)))


def prep(x, edge_index, batch, W1, b1, W2, b2, w_fc, b_fc):
    x = np.asarray(x, dtype=np.float32)
    ei = np.asarray(edge_index).astype(np.int64)
    batch = np.asarray(batch).astype(np.int64)
    W1 = np.asarray(W1, dtype=np.float32)
    b1 = np.asarray(b1, dtype=np.float32)
    W2 = np.asarray(W2, dtype=np.float32)
    b2 = np.asarray(b2, dtype=np.float32)
    w_fc = np.asarray(w_fc, dtype=np.float32)
    b_fc = np.asarray(b_fc, dtype=np.float32)

    n = x.shape[0]
    src, dst = ei[0], ei[1]

    # degree includes self-loop (reference concatenates loops)
    deg = (np.bincount(dst, minlength=n) + 1.0).astype(np.float32)
    dinv = (1.0 / np.sqrt(deg, dtype=np.float32)).astype(np.float32)
    coef = (dinv[src] * dinv[dst]).astype(np.float32)
    selfc = (dinv * dinv).astype(np.float32)

    # graphs -> cores balanced by node count; nodes follow their graph
    gcnt = np.bincount(batch, minlength=G).astype(np.int64)
    bins = _assign_graphs(gcnt)
    core_of_graph = np.zeros(G, dtype=np.int64)
    for c in range(NC):
        core_of_graph[bins[c]] = c
    owner = core_of_graph[batch]                  # owning core per node
    nodes_of = [np.where(owner == c)[0] for c in range(NC)]
    n_c = np.array([len(v) for v in nodes_of])
    # one extra tile of headroom so the packer can keep every tile at 2 chunks
    n_pad = int((int(np.ceil(n_c.max() / 128.0)) + 1) * 128)
    T = n_pad // 128
    TA = _ta(T)

    # Permute nodes within each core's block so that each 128-row dst tile
    # draws <=256 in-edges from every src core (2 chunks per tile).
    vec = np.zeros((n, NC), dtype=np.int64)
    np.add.at(vec, (dst, owner[src]), 1)
    newpos = np.zeros(n, dtype=np.int64)
    for c in range(NC):
        newpos[nodes_of[c]] = _pack_tiles(vec[nodes_of[c]], T)

    gd = owner * n_pad + newpos                   # global padded slot

    # processing order: phase A tiles (t < TA, c-major), then phase B
    order = [(c, t) for rng in (range(TA), range(TA, T))
             for c in range(NC) for t in rng]

    # per-core edge streams (src owned by core), grouped by global dst tile
    per_core = []
    for c in range(NC):
        m = owner[src] == c
        es = newpos[src[m]]                          # local z row (permuted)
        gde = gd[dst[m]]
        ec = coef[m]
        o = np.argsort(gde, kind="stable")
        es, ec, gde = es[o], ec[o], gde[o]
        tb = np.searchsorted(gde, np.arange(0, NC * n_pad + 1, 128))
        per_core.append((es, ec, gde, tb))

    # shared chunk schedule in processing order
    CH = []
    for (c_t) in order:
        cc, tt = c_t
        g = cc * T + tt
        mx = 1
        for c in range(NC):
            tb = per_core[c][3]
            cnt = int(tb[g + 1] - tb[g])
            mx = max(mx, (cnt + 127) // 128)
        CH.append(mx)
    TOT = sum(CH)

    # per-graph 1/count for mean pooling
    gcnt = np.bincount(batch, minlength=G).astype(np.float32)
    ginv = 1.0 / np.maximum(gcnt, 1.0)

    in_maps = []
    for c in range(NC):
        es, ec, gde, tb = per_core[c]
        gsrc = np.zeros(TOT * 128, dtype=np.int16)
        dlv = np.zeros(TOT * 128, dtype=np.float32)
        cfv = np.zeros(TOT * 128, dtype=np.float32)
        off = 0
        for i, (cc, tt) in enumerate(order):
            g = cc * T + tt
            a, b = int(tb[g]), int(tb[g + 1])
            cnt = b - a
            gsrc[off:off + cnt] = es[a:b]
            dlv[off:off + cnt] = (gde[a:b] - g * 128).astype(np.float32)
            cfv[off:off + cnt] = ec[a:b]
            off += CH[i] * 128
        gidx_sb = _wrap16(gsrc, TOT * 8).astype(np.int16)
        dlcf = np.zeros((128, 2 * TOT), dtype=np.float32)
        dlcf[:, :TOT] = dlv.reshape(TOT, 128).T
        dlcf[:, TOT:] = cfv.reshape(TOT, 128).T

        nodes = nodes_of[c]
        pos = newpos[nodes]
        xT = np.zeros((D_IN, n_pad), dtype=BF)
        xT[:, pos] = x[nodes].T.astype(BF)

        sc = np.zeros((128, T), dtype=np.float32)
        sc[pos % 128, pos // 128] = selfc[nodes]

        # local graph slot (0..15) of each node's graph on this core
        slot_of_graph = np.zeros(G, dtype=np.int64)
        slot_of_graph[bins[c]] = np.arange(GP)
        gl = slot_of_graph[batch[nodes]]
        pp = np.zeros((128, T * 16), dtype=np.float32)
        pp[pos % 128, (pos // 128) * 16 + gl] = ginv[batch[nodes]]

        b8 = np.zeros((1, 2 * DH), dtype=BF)
        b8[0, :DH] = (b1 / NC).astype(BF)
        b8[0, DH:] = (b2 / NC).astype(BF)

        in_maps.append({
            "xT": xT,
            "w1": W1.astype(BF),
            "w2": W2.astype(BF),
            "pp": pp.astype(BF),
            "sc": sc,
            "b8": b8,
            "wfc": np.ascontiguousarray(np.broadcast_to(w_fc[:, 0][None, :], (16, DH)).astype(np.float32)),
            "bfc": np.full((16, 1), float(b_fc[0]), dtype=np.float32),
            "gidx": gidx_sb,
            "dlcf": dlcf,
        })

    return in_maps, n_pad, tuple(CH)


# ---------------------------------------------------------------- device build

_CACHE = {}


def build(n_pad, CH):
    key = (n_pad, CH)
    if key in _CACHE:
        return _CACHE[key]
    T = n_pad // 128
    TA = _ta(T)
    TB = T - TA
    TOT = sum(CH)
    GRP = 8  # chunks per dma_gather (1024 idxs; hard per-call limit)

    nc = bacc.Bacc(dynamic_dma_scratch_size=98304)
    xT_in = nc.dram_tensor("xT", [D_IN, n_pad], BF16, kind="ExternalInput")
    w1_in = nc.dram_tensor("w1", [D_IN, DH], BF16, kind="ExternalInput")
    w2_in = nc.dram_tensor("w2", [DH, DH], BF16, kind="ExternalInput")
    pp_in = nc.dram_tensor("pp", [128, T * 16], BF16, kind="ExternalInput")
    sc_in = nc.dram_tensor("sc", [128, T], FP32, kind="ExternalInput")
    b8_in = nc.dram_tensor("b8", [1, 2 * DH], BF16, kind="ExternalInput")
    wfc_in = nc.dram_tensor("wfc", [16, DH], FP32, kind="ExternalInput")
    bfc_in = nc.dram_tensor("bfc", [16, 1], FP32, kind="ExternalInput")
    gidx_in = nc.dram_tensor("gidx", [128, TOT * 8], I16, kind="ExternalInput")
    dlcf_in = nc.dram_tensor("dlcf", [128, 2 * TOT], FP32, kind="ExternalInput")
    out = nc.dram_tensor("out", [16, 1], FP32, kind="ExternalOutput")

    # partial/agg live in partition-major layout: row p of block o holds
    # partition p's features for all tiles of o, so DMA runs are contiguous
    # and the RS shard boundary (dim0 / NC) is exactly one core's block.
    PD = BF16
    zloc = [nc.dram_tensor(f"z{l}loc", [n_pad, DH], BF16) for l in (1, 2)]
    partA = [nc.dram_tensor(f"p{l}A", [NC * 128, TA * DH], PD) for l in (1, 2)]
    partB = [nc.dram_tensor(f"p{l}B", [NC * 128, TB * DH], PD) for l in (1, 2)]
    aggA = [nc.dram_tensor(f"a{l}A", [128, TA * DH], PD) for l in (1, 2)]
    aggB = [nc.dram_tensor(f"a{l}B", [128, TB * DH], PD) for l in (1, 2)]
    zv = [t.rearrange("(t p) f -> p t f", p=128) for t in zloc]
    pvA = [t.rearrange("(o p) (t f) -> o p t f", p=128, f=DH) for t in partA]
    pvB = [t.rearrange("(o p) (t f) -> o p t f", p=128, f=DH) for t in partB]
    avA = [t.rearrange("p (t f) -> p t f", f=DH) for t in aggA]
    avB = [t.rearrange("p (t f) -> p t f", f=DH) for t in aggB]

    with TileContext(nc) as tc:
        with (
            tc.tile_pool(name="const", bufs=1) as const,
            tc.tile_pool(name="gp", bufs=5) as gp,
            tc.tile_pool(name="sp", bufs=12) as sp,
            tc.tile_pool(name="pw", bufs=6) as pw,
            tc.tile_pool(name="ap", bufs=3) as app,
            tc.tile_pool(name="hp", bufs=8) as hp,
            tc.tile_pool(name="smp", bufs=8) as smp,
            tc.tile_pool(name="tp", bufs=4) as tp,
            tc.tile_pool(name="fp", bufs=1) as fp,
            tc.tile_pool(name="psA", bufs=2, space="PSUM") as psA,
            tc.tile_pool(name="psM", bufs=3, space="PSUM") as psM,
            tc.tile_pool(name="psT", bufs=2, space="PSUM") as psT,
            tc.tile_pool(name="psP", bufs=1, space="PSUM") as psP,
        ):
            # ---- constant loads
            xT_sb = const.tile([128, 4, n_pad], BF16)
            for k in range(4):
                nc.sync.dma_start(out=xT_sb[:, k, :], in_=xT_in[k * 128:(k + 1) * 128, :])
            w1_sb = const.tile([128, 4, DH], BF16)
            for k in range(4):
                nc.sync.dma_start(out=w1_sb[:, k, :], in_=w1_in[k * 128:(k + 1) * 128, :])
            w2_sb = const.tile([128, 2, DH], BF16)
            for k in range(2):
                nc.sync.dma_start(out=w2_sb[:, k, :], in_=w2_in[k * 128:(k + 1) * 128, :])
            gidx_sb = const.tile([128, TOT * 8], I16)
            nc.sync.dma_start(out=gidx_sb[:], in_=gidx_in[:])
            dlcf_sb = const.tile([128, 2 * TOT], FP32)
            nc.sync.dma_start(out=dlcf_sb[:], in_=dlcf_in[:])
            pp_sb = const.tile([128, T * 16], BF16)
            nc.sync.dma_start(out=pp_sb[:], in_=pp_in[:])
            sc_sb = const.tile([128, T], FP32)
            nc.sync.dma_start(out=sc_sb[:], in_=sc_in[:])
            b8_sb = const.tile([1, 2 * DH], BF16)
            nc.sync.dma_start(out=b8_sb[:], in_=b8_in[:])
            wfc_sb = const.tile([16, DH], FP32)
            nc.sync.dma_start(out=wfc_sb[:], in_=wfc_in[:])
            bfc_sb = const.tile([16, 1], FP32)
            nc.sync.dma_start(out=bfc_sb[:], in_=bfc_in[:])

            ident = const.tile([128, 128], BF16)
            make_identity(nc, ident[:])
            ones_sb = const.tile([1, 128], BF16)
            nc.vector.memset(ones_sb[:], 1.0)
            iota_i = const.tile([128, 128], I32)
            nc.gpsimd.iota(iota_i[:], pattern=[[1, 128]], base=0, channel_multiplier=0)
            iota_f = const.tile([128, 128], BF16)
            nc.vector.tensor_copy(iota_f[:], iota_i[:])

            # persistent z (bf16) per layer, also staging for DRAM writes
            z1sb = const.tile([128, T, DH], BF16)
            z2sb = const.tile([128, T, DH], BF16)
            zsb = [z1sb, z2sb]

            # ---- phase A: z1 = x @ W1 (own nodes)
            for t in range(T):
                acc = psA.tile([128, DH], FP32, space="PSUM", tag="psA")
                for k in range(4):
                    nc.tensor.matmul(
                        out=acc[:], lhsT=xT_sb[:, k, t * 128:(t + 1) * 128],
                        rhs=w1_sb[:, k, :], start=(k == 0), stop=(k == 3))
                nc.vector.tensor_copy(zsb[0][:, t, :], acc[:])
            for w in range(0, T, 8):
                e = min(T, w + 8)
                nc.sync.dma_start(out=zv[0][:, w:e, :], in_=zsb[0][:, w:e, :])
            # pre-scale z by self-loop coef in place (post-RS adds it directly)
            for t in range(T):
                nc.vector.tensor_scalar_mul(
                    zsb[0][:, t, :], zsb[0][:, t, :], sc_sb[:, t:t + 1])

            # ---- two GCN layers
            pool_holder = [None]
            for l in range(2):
                gtile = [None, None]  # (group id, tile)

                def get_msg(q, l=l, gtile=gtile):
                    grp = q // GRP
                    if gtile[0] != grp:
                        sz = min(GRP, TOT - grp * GRP)
                        gt = gp.tile([128, sz, DH], BF16, tag="g")
                        nc.gpsimd.dma_gather(
                            out_ap=gt[:],
                            in_ap=zloc[l][:],
                            idxs_ap=gidx_sb[:, grp * GRP * 8:(grp * GRP + sz) * 8],
                            num_idxs=sz * 128,
                            num_idxs_reg=sz * 128,
                            elem_size=DH,
                        )
                        gtile[0], gtile[1] = grp, gt
                    return gtile[1][:, q % GRP, :]

                q0 = 0   # global chunk counter (layer-local)
                i0 = 0   # processing tile counter

                def scatter_phase(PT, pview, q0, i0, l=l, get_msg=get_msg,
                                  emit_mid=None, mid_at=8):
                    # processing order: dst block o major, local tile t minor;
                    # pstage groups of 4 tiles never cross a block boundary
                    pstage = None
                    off = q0
                    for i in range(NC * PT):
                        if emit_mid is not None and i == mid_at:
                            emit_mid()
                        o, t = i // PT, i % PT
                        ch = CH[i0 + i]
                        acc = psM.tile([128, DH], FP32, space="PSUM", tag="psM")
                        for j in range(ch):
                            q = off + j
                            msg = get_msg(q)
                            S = sp.tile([128, 128], BF16, tag="S")
                            nc.vector.tensor_scalar(
                                out=S[:], in0=iota_f[:],
                                scalar1=dlcf_sb[:, q:q + 1],
                                scalar2=dlcf_sb[:, TOT + q:TOT + q + 1],
                                op0=mybir.AluOpType.is_equal,
                                op1=mybir.AluOpType.mult)
                            nc.tensor.matmul(out=acc[:], lhsT=S[:], rhs=msg,
                                             start=(j == 0), stop=False)
                        nc.tensor.matmul(
                            out=acc[:], lhsT=ones_sb[:],
                            rhs=b8_sb[0:1, l * DH:(l + 1) * DH],
                            start=False, stop=True)
                        if t % 4 == 0:
                            pstage = pw.tile([128, 4, DH], PD, tag="pst")
                        nc.scalar.activation(out=pstage[:, t % 4, :], in_=acc[:],
                                             func=mybir.ActivationFunctionType.Copy)
                        if t % 4 == 3 or t == PT - 1:
                            w0 = (t // 4) * 4
                            nc.sync.dma_start(out=pview[o, :, w0:t + 1, :],
                                              in_=pstage[:, :t + 1 - w0, :])
                        off += ch
                    return off

                def post_phase(t_lo, t_hi, aview, l=l):
                    # engine-stage order inside each group so tiles pipeline
                    # instead of serializing on cross-engine semaphores
                    for w in range(t_lo, t_hi, 8):
                        e = min(t_hi, w + 8)
                        ast = app.tile([128, 8, DH], PD, tag="agg")
                        nc.sync.dma_start(out=ast[:, :e - w, :],
                                          in_=aview[:, w - t_lo:e - t_lo, :])
                        hs = []
                        for t in range(w, e):
                            sm = smp.tile([128, DH], BF16, tag="sm")
                            nc.vector.tensor_tensor(out=sm[:], in0=zsb[l][:, t, :],
                                                    in1=ast[:, t - w, :],
                                                    op=mybir.AluOpType.add)
                            h = hp.tile([128, DH], BF16, tag="h")
                            nc.vector.tensor_scalar_max(h[:], sm[:], 0.0)
                            hs.append(h)
                        for t in range(w, e):
                            h = hs[t - w]
                            if l == 0:
                                hT = []
                                for half in range(2):
                                    pt = psT.tile([128, 128], BF16, space="PSUM", tag="psT")
                                    nc.tensor.transpose(
                                        out=pt[:], in_=h[:, half * 128:(half + 1) * 128],
                                        identity=ident[:])
                                    ht = tp.tile([128, 128], BF16, tag="hT")
                                    nc.vector.tensor_copy(ht[:], pt[:])
                                    hT.append(ht)
                                accz = psA.tile([128, DH], FP32, space="PSUM", tag="psA")
                                for half in range(2):
                                    nc.tensor.matmul(out=accz[:], lhsT=hT[half][:],
                                                     rhs=w2_sb[:, half, :],
                                                     start=(half == 0), stop=(half == 1))
                                nc.vector.tensor_copy(zsb[1][:, t, :], accz[:])
                            else:
                                if t == 0:
                                    pool_holder[0] = psP.tile([16, DH], FP32, space="PSUM",
                                                              tag="psP", name="pool_acc")
                                pool_acc = pool_holder[0]
                                nc.tensor.matmul(out=pool_acc[:],
                                                 lhsT=pp_sb[:, t * 16:(t + 1) * 16],
                                                 rhs=h[:], start=(t == 0), stop=(t == T - 1),
                                                 skip_group_check=True)
                        if l == 0:
                            nc.sync.dma_start(out=zv[1][:, w:e, :],
                                              in_=zsb[1][:, w:e, :])

                # phase A scatter; RS-A is emitted a few tiles into the B
                # stream so its sequencer wait doesn't stall the B gathers
                q0 = scatter_phase(TA, pvA[l], q0, 0)

                def emit_rsA(l=l):
                    nc.gpsimd.collective_compute(
                        "ReduceScatter", mybir.AluOpType.add,
                        ins=[partA[l][:]], outs=[aggA[l][:]],
                        replica_groups=[list(range(NC))])

                # phase B scatter + RS-B
                q0 = scatter_phase(TB, pvB[l], q0, NC * TA,
                                   emit_mid=emit_rsA, mid_at=4)
                nc.gpsimd.collective_compute(
                    "ReduceScatter", mybir.AluOpType.add,
                    ins=[partB[l][:]], outs=[aggB[l][:]],
                    replica_groups=[list(range(NC))])
                # post-RS work (A overlaps RS-B). The fence keeps the agg
                # loads (which wait on the collectives) from being scheduled
                # ahead of the B-phase partial writes on the SP queue.
                tc.no_sync_barrier()
                post_phase(0, TA, avA[l])
                post_phase(TA, T, avB[l])
                if l == 0:
                    # pre-scale z2 by self coef in place (off the critical
                    # path; runs during the layer-2 scatter ramp)
                    for t in range(T):
                        nc.vector.tensor_scalar_mul(
                            zsb[1][:, t, :], zsb[1][:, t, :], sc_sb[:, t:t + 1])

            # ---- fc head: out = pooled @ w_fc + b_fc
            pooled = fp.tile([16, DH], FP32)
            nc.vector.tensor_copy(pooled[:], pool_holder[0][:])
            prod = fp.tile([16, DH], FP32)
            nc.vector.tensor_tensor(out=prod[:], in0=pooled[:], in1=wfc_sb[:],
                                    op=mybir.AluOpType.mult)
            red = fp.tile([16, 1], FP32)
            nc.vector.reduce_sum(red[:], prod[:], axis=mybir.AxisListType.X)
            outv = fp.tile([16, 1], FP32)
            nc.vector.tensor_scalar_add(outv[:], red[:], bfc_sb[:])
            nc.sync.dma_start(out=out[:], in_=outv[:])

    nc.finalize()
    _CACHE[key] = nc
    return nc


# ---------------------------------------------------------------- entry points

def _run(inputs, trace=False):
    in_maps, n_pad, CH = prep(**inputs)
    nc = build(n_pad, CH)
    r = run_bass_kernel_spmd(nc, in_maps, list(range(NC)), trace=trace)
    batch = np.asarray(inputs["batch"]).astype(np.int64)
    bins = _assign_graphs(np.bincount(batch, minlength=G).astype(np.int64))
    out = np.zeros(G, dtype=np.float32)
    for c in range(NC):
        out[bins[c]] = r.results[c]["out"][:, 0]
    return out, r


def kernel(**inputs):
    out, _ = _run(inputs, trace=False)
    return out


def kernel_traced(**inputs):
    out, r = _run(inputs, trace=True)
    return out, r
